# revision 14
# baseline (speedup 1.0000x reference)
"""Multi-head attention (B=2, C=256, N=64*64=4096, heads=8, d=32) on 8 trn2
NeuronCores via Bass/Tile.

Sharding: batch x head-pair. Core c handles batch c//4 and heads
(2*(c%4), 2*(c%4)+1). Each core computes the full NxN attention for its two
heads plus its partial contribution to the output projection; the host sums
the 4 per-batch partials (each core adds b_out/4 so the sum carries the bias
exactly once).

Per-core pipeline (matmuls in f32r = 1 cycle/row on the PE):
  1. QKV: Q^T,K^T per head in 4x-replicated [128,4096] layout (replication via
     host-replicated weight columns) so the NxN score matmuls can use PE
     row-tiling at K=32. V in [token, d] layout with an appended ones column:
     the attention-weight matmul then produces the softmax denominator as an
     extra output row for free. Emitted K -> V -> Q so the first attention
     wave can start as early as possible; Q/K bias drains run on the (then
     idle) ScalarE.
  2. Waves (per head, per 512-wide i-block, per 3 j-blocks): 3 row-packed
     matmuls -> S^T in a 3-bank PSUM slot -> one ScalarE exp (PSUM->SBUF,
     f32r out) -> 3 AV matmuls (M=33) accumulate out^T + the denominator row
     into a dedicated PSUM bank held across the whole i-block. Emission is
     software-pipelined (S^T of wave w+1 is issued before AV of wave w) so
     the in-order PE never sits waiting for exp. ScalarE's exp is the
     per-core roofline (~1.6us/wave).
  3. Per i-block: copy the AV bank to SBUF, reciprocal of the denominator
     rows, K=1-matmul broadcast across 32 partitions, normalize, per-head
     projection matmuls in y^T = [c_out, i] layout (matches the [B,C,H,W]
     output layout), add b_out/4, DMA out.
"""

import sys

sys.path.insert(0, "/opt/trn_rl_repo")

import numpy as np

B, C, HW, N = 2, 256, 64, 4096
HEADS, D = 8, 32
SCALE = float(D) ** -0.5
NCORES = 8
IB = 512            # i-block width
N_IB = N // IB      # 8
JB = 128            # j-block height
N_JB = N // JB      # 32
WAVE_JB = 3         # j-blocks per wave
N_WAVE = (N_JB + WAVE_JB - 1) // WAVE_JB  # 11 (10x3 + 1x2)

_COMPILED = None


def _build():
    from concourse import bacc, mybir
    from concourse.tile import TileContext

    dt = mybir.dt
    f32, f32r, bf16 = dt.float32, dt.float32r, dt.bfloat16
    Exp = mybir.ActivationFunctionType.Exp
    ADD = mybir.AluOpType.add
    MULT = mybir.AluOpType.mult

    nc = bacc.Bacc("TRN2", target_bir_lowering=False, debug=False,
                   enable_asserts=True, num_devices=NCORES)

    xb = nc.dram_tensor("xb", [C, N], f32, kind="ExternalInput").ap()
    wq_rep = nc.dram_tensor("wq_rep", [2, C, 128], f32, kind="ExternalInput").ap()
    wk_rep = nc.dram_tensor("wk_rep", [2, C, 128], f32, kind="ExternalInput").ap()
    wv = nc.dram_tensor("wv", [C, 2 * D], f32, kind="ExternalInput").ap()
    bq_rep = nc.dram_tensor("bq_rep", [2, 128], f32, kind="ExternalInput").ap()
    bk_rep = nc.dram_tensor("bk_rep", [2, 128], f32, kind="ExternalInput").ap()
    bv_rep = nc.dram_tensor("bv_rep", [128, 2 * D], f32, kind="ExternalInput").ap()
    wo = nc.dram_tensor("wo", [D, 2 * C], f32, kind="ExternalInput").ap()
    bo4 = nc.dram_tensor("bo4", [C], f32, kind="ExternalInput").ap()
    o = nc.dram_tensor("o", [C, N], f32, kind="ExternalOutput").ap()

    with TileContext(nc) as tc:
        with tc.tile_pool(name="const", bufs=1) as cp, \
             tc.tile_pool(name="big", bufs=1) as bp, \
             tc.tile_pool(name="xtmp", bufs=2) as xp, \
             tc.tile_pool(name="work", bufs=2) as wkp, \
             tc.tile_pool(name="ewp", bufs=4) as ep, \
             tc.tile_pool(name="ps", bufs=2, space="PSUM") as ps:

            # ---- load + f32r-round weights/biases -------------------------
            def load_rounded(name, src, shape):
                raw = cp.tile(shape, f32, name=name + "_f", tag=name + "_f")
                nc.gpsimd.dma_start(out=raw[:], in_=src)
                rnd = cp.tile(shape, f32r, name=name + "_r", tag=name + "_r")
                nc.vector.tensor_copy(out=rnd[:], in_=raw[:])
                return rnd

            wq_r = [[load_rounded(f"wq{h}{cc}", wq_rep[h, 128 * cc:128 * (cc + 1), :], [128, 128])
                     for cc in range(2)] for h in range(2)]
            wk_r = [[load_rounded(f"wk{h}{cc}", wk_rep[h, 128 * cc:128 * (cc + 1), :], [128, 128])
                     for cc in range(2)] for h in range(2)]
            wv_r = [load_rounded(f"wv{cc}", wv[128 * cc:128 * (cc + 1), :], [128, 2 * D])
                    for cc in range(2)]
            wo_r = load_rounded("wo", wo[:, :], [D, 2 * C])

            bq_sb = cp.tile([128, 2], f32, name="bq_sb", tag="bq_sb")
            bk_sb = cp.tile([128, 2], f32, name="bk_sb", tag="bk_sb")
            for h in range(2):
                nc.sync.dma_start(out=bq_sb[:, h:h + 1], in_=bq_rep[h, :])
                nc.sync.dma_start(out=bk_sb[:, h:h + 1], in_=bk_rep[h, :])
            bv_sb = cp.tile([128, 2 * D], f32, name="bv_sb", tag="bv_sb")
            nc.sync.dma_start(out=bv_sb[:], in_=bv_rep[:, :])
            bo_sb = cp.tile([128, 2], f32, name="bo_sb", tag="bo_sb")
            for cc in range(2):
                nc.sync.dma_start(out=bo_sb[:, cc:cc + 1], in_=bo4[128 * cc:128 * (cc + 1)])

            # ---- PE warm-up: ~6us of dummy matmuls so HAM reaches 2.4GHz
            # while the x DMAs are still in flight ------------------------
            warm_t = cp.tile([128, 512], f32r, name="warm_t", tag="warm_t")
            nc.vector.memset(warm_t[:].bitcast(f32), 1.0)
            wslot = ps.tile([128, 512], f32, tag="av", name="warm_ps")
            for _ in range(12):
                nc.tensor.matmul(out=wslot[:, :], lhsT=warm_t[:, 0:128], rhs=warm_t[:, :],
                                 start=True, stop=True)

            # ---- x -> SBUF, round to f32r ---------------------------------
            x_r = []
            for cc in range(2):
                xr = bp.tile([128, N], f32r, name=f"x_r{cc}", tag=f"x_r{cc}")
                x_raw = xp.tile([128, N], f32, tag="xraw", name="x_raw")
                eng = (nc.sync, nc.scalar)[cc]
                eng.dma_start(out=x_raw[:], in_=xb[128 * cc:128 * (cc + 1), :])
                for piece in range(2):
                    sl = slice(piece * (N // 2), (piece + 1) * (N // 2))
                    nc.vector.tensor_copy(out=xr[:, sl], in_=x_raw[:, sl])
                x_r.append(xr)

            # ---- QKV projections: K, then V, then Q -----------------------
            qt = [bp.tile([128, N], f32r, name=f"qt{h}", tag=f"qt{h}") for h in range(2)]
            kt = [bp.tile([128, N], f32r, name=f"kt{h}", tag=f"kt{h}") for h in range(2)]

            def qk_phase(dst, w_r, b_sb, h):
                # 8 i-blocks, 3 per [128,1536] psum slot
                for g0 in range(0, N_IB, 3):
                    blks = list(range(g0, min(g0 + 3, N_IB)))
                    slot = ps.tile([128, len(blks) * IB], f32, tag="ps3", name="qk_ps")
                    for bi, ib in enumerate(blks):
                        for cc in range(2):
                            nc.tensor.matmul(
                                out=slot[:, bi * IB:(bi + 1) * IB],
                                lhsT=w_r[h][cc][:, :],
                                rhs=x_r[cc][:, ib * IB:(ib + 1) * IB],
                                start=(cc == 0), stop=(cc == 1))
                    dsl = dst[h][:, g0 * IB:(g0 + len(blks)) * IB]
                    if h == 0:
                        # ScalarE is idle before the first wave
                        nc.scalar.add(out=dsl, in_=slot[:, :], add=b_sb[:, h:h + 1])
                    else:
                        # DVE has slack once waves are running
                        nc.vector.tensor_scalar_add(out=dsl, in0=slot[:, :],
                                                    scalar1=b_sb[:, h:h + 1])

            # V in [token, d] layout + ones columns: [128, 32jb, 66]
            v_sb = bp.tile([128, N_JB, 66], bf16, name="v_sb", tag="v_sb")
            nc.vector.memset(v_sb[:, :, 32:33], 1.0)
            nc.vector.memset(v_sb[:, :, 65:66], 1.0)
            for grp in range(N_JB // 4):  # 4 j-blocks per psum slot
                vslot = ps.tile([128, 4, 64], f32, tag="av", name="v_ps")
                for j in range(4):
                    jb = grp * 4 + j
                    for cc in range(2):
                        nc.tensor.matmul(
                            out=vslot[:, j, :],
                            lhsT=x_r[cc][:, jb * JB:(jb + 1) * JB],
                            rhs=wv_r[cc][:, :],
                            start=(cc == 0), stop=(cc == 1))
                for j in range(4):
                    jb = grp * 4 + j
                    nc.vector.scalar_tensor_tensor(
                        out=v_sb[:, jb, 0:32], in0=vslot[:, j, 0:32], scalar=1.0,
                        in1=bv_sb[:, 0:32], op0=MULT, op1=ADD)
                    nc.vector.scalar_tensor_tensor(
                        out=v_sb[:, jb, 33:65], in0=vslot[:, j, 32:64], scalar=1.0,
                        in1=bv_sb[:, 32:64], op0=MULT, op1=ADD)


            for h in range(2):
                qk_phase(kt, wk_r, bk_sb, h)
                qk_phase(qt, wq_r, bq_sb, h)

            # ---- attention waves + projection -----------------------------
            # phase3(n) is emitted between i-block n+1's two head passes so
            # the PE never idles on the reciprocal chain (keeps HAM warm).
            pending_p3 = [None]

            def flush_p3():
                if pending_p3[0] is not None:
                    pending_p3[0]()
                    pending_p3[0] = None

            for n in range(N_IB):
                i0 = n * IB
                acc = wkp.tile([33, 1024], f32, tag="acc", name="acc")
                for h in range(2):
                    if h == 1:
                        flush_p3()
                    av = ps.tile([33, IB], f32, tag="av", name="av_ps")
                    prev = None  # (ew, jbs) of previous wave

                    def emit_av(prev):
                        ew_p, jbs_p = prev
                        for r, jb in enumerate(jbs_p):
                            nc.tensor.matmul(
                                out=av[:, :],
                                lhsT=v_sb[:, jb, 33 * h:33 * h + 33],
                                rhs=ew_p[:, r * IB:(r + 1) * IB],
                                start=(jb == 0), stop=(jb == N_JB - 1),
                                tile_position=(0, 0))

                    for w in range(N_WAVE):
                        jbs = list(range(w * WAVE_JB, min((w + 1) * WAVE_JB, N_JB)))
                        slot = ps.tile([128, len(jbs) * IB], f32, tag="ps3", name="wave_ps")
                        for r, jb in enumerate(jbs):
                            nc.tensor.matmul(
                                out=slot[:, r * IB:(r + 1) * IB],
                                lhsT=kt[h][32 * r:32 * r + 32, jb * JB:(jb + 1) * JB],
                                rhs=qt[h][32 * r:32 * r + 32, i0:i0 + IB],
                                start=True, stop=True, tile_position=(32 * r, 0))
                        if prev is not None:
                            emit_av(prev)
                        ew = ep.tile([128, len(jbs) * IB], bf16, tag="ew", name="ew")
                        nc.scalar.activation(out=ew[:], in_=slot[:, :], func=Exp, scale=SCALE)
                        prev = (ew, jbs)
                    emit_av(prev)
                    nc.vector.tensor_copy(out=acc[:, h * IB:(h + 1) * IB], in_=av[:, :])

                # projection for this i-block; reciprocal right away (DVE),
                # PE-side matmuls deferred into the next i-block's waves
                recip = wkp.tile([33, 1024], f32, tag="recip", name="recip", bufs=1)
                nc.vector.tensor_copy(out=recip[0:1, :], in_=acc[32:33, :])
                recipf = wkp.tile([1, 1024], f32, tag="recipf", name="recipf", bufs=1)
                nc.vector.reciprocal_approx_fast(out=recipf[0:1, :], in_=recip[0:1, :])
                bc_sb = wkp.tile([32, 1024], f32, tag="bc_sb", name="bc_sb", bufs=1)
                nc.gpsimd.partition_broadcast(bc_sb[:, :], recipf[0:1, :])

                def phase3(n=n, i0=i0, acc=acc, bc_sb=bc_sb):
                    norm = wkp.tile([32, 1024], f32r, tag="norm", name="norm", bufs=1)
                    nc.vector.tensor_tensor(out=norm[:, :], in0=bc_sb[:, :],
                                            in1=acc[0:32, :], op=MULT)
                    for cc in range(2):
                        pj = ps.tile([128, IB], f32, tag="av", name="pj_ps")
                        for h in range(2):
                            nc.tensor.matmul(
                                out=pj[:, :],
                                lhsT=wo_r[:, h * C + cc * 128: h * C + (cc + 1) * 128],
                                rhs=norm[:, h * IB:(h + 1) * IB],
                                start=(h == 0), stop=(h == 1), tile_position=(0, 0))
                        y = wkp.tile([128, IB], f32, tag="y", name="y")
                        nc.vector.tensor_scalar_add(out=y[:], in0=pj[:, :],
                                                    scalar1=bo_sb[:, cc:cc + 1])
                        nc.sync.dma_start(out=o[cc * 128:(cc + 1) * 128, i0:i0 + IB], in_=y[:])

                pending_p3[0] = phase3
            flush_p3()

    nc.finalize()
    return nc


def _get_compiled():
    global _COMPILED
    if _COMPILED is None:
        _COMPILED = _build()
    return _COMPILED


def _make_in_maps(x, w_qkv, b_qkv, w_out, b_out):
    x = np.asarray(x, dtype=np.float32)
    w_qkv = np.asarray(w_qkv, dtype=np.float32)
    b_qkv = np.asarray(b_qkv, dtype=np.float32)
    w_out = np.asarray(w_out, dtype=np.float32)
    b_out = np.asarray(b_out, dtype=np.float32)

    xf = x.reshape(B, C, N)
    in_maps = []
    for core in range(NCORES):
        b = core // 4
        q = core % 4
        heads = (2 * q, 2 * q + 1)

        def rep_cols(w_slice):  # [C,32] -> [C,128] (4 replicas)
            return np.ascontiguousarray(np.tile(w_slice, (1, 4)))

        wq = np.stack([rep_cols(w_qkv[:, 32 * h:32 * h + 32]) for h in heads])
        wk = np.stack([rep_cols(w_qkv[:, C + 32 * h:C + 32 * h + 32]) for h in heads])
        wv_ = np.concatenate([w_qkv[:, 2 * C + 32 * h:2 * C + 32 * h + 32] for h in heads], axis=1)
        bq = np.stack([np.tile(b_qkv[32 * h:32 * h + 32], 4) for h in heads])
        bk = np.stack([np.tile(b_qkv[C + 32 * h:C + 32 * h + 32], 4) for h in heads])
        bv = np.concatenate([b_qkv[2 * C + 32 * h:2 * C + 32 * h + 32] for h in heads])
        bv_rep = np.tile(bv[None, :], (128, 1))
        wo_ = np.concatenate([w_out[32 * h:32 * h + 32, :] for h in heads], axis=1)
        in_maps.append({
            "xb": np.ascontiguousarray(xf[b]),
            "wq_rep": wq,
            "wk_rep": wk,
            "wv": np.ascontiguousarray(wv_),
            "bq_rep": np.ascontiguousarray(bq),
            "bk_rep": np.ascontiguousarray(bk),
            "bv_rep": np.ascontiguousarray(bv_rep),
            "wo": np.ascontiguousarray(wo_),
            "bo4": np.ascontiguousarray(b_out / 4.0),
        })
    return in_maps


def kernel(x, w_qkv, b_qkv, w_out, b_out, _trace=False, _trace_kwargs=None):
    from concourse.bass_utils import run_bass_kernel_spmd

    nc = _get_compiled()
    in_maps = _make_in_maps(x, w_qkv, b_qkv, w_out, b_out)
    res = run_bass_kernel_spmd(nc, in_maps, list(range(NCORES)),
                               trace=_trace, **(_trace_kwargs or {}))
    parts = [res.results[c]["o"] for c in range(NCORES)]
    out = np.empty((B, C, N), dtype=np.float32)
    for b in range(B):
        out[b] = parts[4 * b] + parts[4 * b + 1] + parts[4 * b + 2] + parts[4 * b + 3]
    result = out.reshape(B, C, HW, HW)
    if _trace:
        return result, res
    return result


# revision 16
# speedup vs baseline: 1.0079x; 1.0079x over previous
"""Multi-head attention (B=2, C=256, N=64*64=4096, heads=8, d=32) on 8 trn2
NeuronCores via Bass/Tile.

Sharding: batch x head-pair. Core c handles batch c//4 and heads
(2*(c%4), 2*(c%4)+1). Each core computes the full NxN attention for its two
heads plus its partial contribution to the output projection; the host sums
the 4 per-batch partials (each core adds b_out/4 so the sum carries the bias
exactly once).

Per-core pipeline (matmuls in f32r = 1 cycle/row on the PE):
  1. QKV: Q^T,K^T per head in 4x-replicated [128,4096] layout (replication via
     host-replicated weight columns) so the NxN score matmuls can use PE
     row-tiling at K=32. V in [token, d] layout with an appended ones column:
     the attention-weight matmul then produces the softmax denominator as an
     extra output row for free. Emitted K -> V -> Q so the first attention
     wave can start as early as possible; Q/K bias drains run on the (then
     idle) ScalarE.
  2. Waves (per head, per 512-wide i-block, per 3 j-blocks): 3 row-packed
     matmuls -> S^T in a 3-bank PSUM slot -> one ScalarE exp (PSUM->SBUF,
     f32r out) -> 3 AV matmuls (M=33) accumulate out^T + the denominator row
     into a dedicated PSUM bank held across the whole i-block. Emission is
     software-pipelined (S^T of wave w+1 is issued before AV of wave w) so
     the in-order PE never sits waiting for exp. ScalarE's exp is the
     per-core roofline (~1.6us/wave).
  3. Per i-block: copy the AV bank to SBUF, reciprocal of the denominator
     rows, K=1-matmul broadcast across 32 partitions, normalize, per-head
     projection matmuls in y^T = [c_out, i] layout (matches the [B,C,H,W]
     output layout), add b_out/4, DMA out.
"""

import sys

sys.path.insert(0, "/opt/trn_rl_repo")

import numpy as np

B, C, HW, N = 2, 256, 64, 4096
HEADS, D = 8, 32
SCALE = float(D) ** -0.5
NCORES = 8
IB = 512            # i-block width
N_IB = N // IB      # 8
JB = 128            # j-block height
N_JB = N // JB      # 32
WAVE_JB = 3         # j-blocks per wave
N_WAVE = (N_JB + WAVE_JB - 1) // WAVE_JB  # 11 (10x3 + 1x2)

_COMPILED = None


def _build():
    from concourse import bacc, mybir
    from concourse.tile import TileContext

    dt = mybir.dt
    f32, f32r, bf16 = dt.float32, dt.float32r, dt.bfloat16
    Exp = mybir.ActivationFunctionType.Exp
    ADD = mybir.AluOpType.add
    MULT = mybir.AluOpType.mult

    nc = bacc.Bacc("TRN2", target_bir_lowering=False, debug=False,
                   enable_asserts=True, num_devices=NCORES)

    xb = nc.dram_tensor("xb", [C, N], f32, kind="ExternalInput").ap()
    wq_rep = nc.dram_tensor("wq_rep", [2, C, 128], f32, kind="ExternalInput").ap()
    wk_rep = nc.dram_tensor("wk_rep", [2, C, 128], f32, kind="ExternalInput").ap()
    wv = nc.dram_tensor("wv", [C, 2 * D], f32, kind="ExternalInput").ap()
    bq_rep = nc.dram_tensor("bq_rep", [2, 128], f32, kind="ExternalInput").ap()
    bk_rep = nc.dram_tensor("bk_rep", [2, 128], f32, kind="ExternalInput").ap()
    bv_rep = nc.dram_tensor("bv_rep", [128, 2 * D], f32, kind="ExternalInput").ap()
    wo = nc.dram_tensor("wo", [D, 2 * C], f32, kind="ExternalInput").ap()
    bo4 = nc.dram_tensor("bo4", [C], f32, kind="ExternalInput").ap()
    o = nc.dram_tensor("o", [C, N], f32, kind="ExternalOutput").ap()

    with TileContext(nc) as tc:
        with tc.tile_pool(name="const", bufs=1) as cp, \
             tc.tile_pool(name="big", bufs=1) as bp, \
             tc.tile_pool(name="xtmp", bufs=2) as xp, \
             tc.tile_pool(name="work", bufs=2) as wkp, \
             tc.tile_pool(name="ewp", bufs=4) as ep, \
             tc.tile_pool(name="ps", bufs=2, space="PSUM") as ps:

            # ---- x -> SBUF (3 DMA queues), round to f32r ------------------
            x_r = []
            for cc in range(2):
                xr = bp.tile([128, N], f32r, name=f"x_r{cc}", tag=f"x_r{cc}")
                x_raw = xp.tile([128, N], f32, tag="xraw", name="x_raw")
                if cc == 0:
                    nc.sync.dma_start(out=x_raw[:], in_=xb[0:128, :])
                else:
                    nc.scalar.dma_start(out=x_raw[:, 0:N // 2], in_=xb[128:256, 0:N // 2])
                    nc.gpsimd.dma_start(out=x_raw[:, N // 2:], in_=xb[128:256, N // 2:])
                for piece in range(2):
                    sl = slice(piece * (N // 2), (piece + 1) * (N // 2))
                    nc.vector.tensor_copy(out=xr[:, sl], in_=x_raw[:, sl])
                x_r.append(xr)

            # ---- load + f32r-round weights/biases -------------------------
            def load_rounded(name, src, shape):
                raw = cp.tile(shape, f32, name=name + "_f", tag=name + "_f")
                nc.gpsimd.dma_start(out=raw[:], in_=src)
                rnd = cp.tile(shape, f32r, name=name + "_r", tag=name + "_r")
                nc.vector.tensor_copy(out=rnd[:], in_=raw[:])
                return rnd

            wq_r = [[load_rounded(f"wq{h}{cc}", wq_rep[h, 128 * cc:128 * (cc + 1), :], [128, 128])
                     for cc in range(2)] for h in range(2)]
            wk_r = [[load_rounded(f"wk{h}{cc}", wk_rep[h, 128 * cc:128 * (cc + 1), :], [128, 128])
                     for cc in range(2)] for h in range(2)]
            wv_r = [load_rounded(f"wv{cc}", wv[128 * cc:128 * (cc + 1), :], [128, 2 * D])
                    for cc in range(2)]
            wo_r = load_rounded("wo", wo[:, :], [D, 2 * C])

            bq_sb = cp.tile([128, 2], f32, name="bq_sb", tag="bq_sb")
            bk_sb = cp.tile([128, 2], f32, name="bk_sb", tag="bk_sb")
            for h in range(2):
                nc.sync.dma_start(out=bq_sb[:, h:h + 1], in_=bq_rep[h, :])
                nc.sync.dma_start(out=bk_sb[:, h:h + 1], in_=bk_rep[h, :])
            bv_sb = cp.tile([128, 2 * D], f32, name="bv_sb", tag="bv_sb")
            nc.sync.dma_start(out=bv_sb[:], in_=bv_rep[:, :])
            bo_sb = cp.tile([128, 2], f32, name="bo_sb", tag="bo_sb")
            for cc in range(2):
                nc.sync.dma_start(out=bo_sb[:, cc:cc + 1], in_=bo4[128 * cc:128 * (cc + 1)])

            # ---- PE warm-up: ~6us of dummy matmuls so HAM reaches 2.4GHz
            # while the x DMAs are still in flight ------------------------
            warm_t = cp.tile([128, 512], f32r, name="warm_t", tag="warm_t")
            nc.vector.memset(warm_t[:].bitcast(f32), 1.0)
            wslot = ps.tile([128, 512], f32, tag="av", name="warm_ps")
            for _ in range(12):
                nc.tensor.matmul(out=wslot[:, :], lhsT=warm_t[:, 0:128], rhs=warm_t[:, :],
                                 start=True, stop=True)

            # ---- QKV projections: K, then V, then Q -----------------------
            qt = [bp.tile([128, N], f32r, name=f"qt{h}", tag=f"qt{h}") for h in range(2)]
            kt = [bp.tile([128, N], f32r, name=f"kt{h}", tag=f"kt{h}") for h in range(2)]

            def v_phase():
                nc.vector.memset(v_sb[:, :, 32:33], 1.0)
                nc.vector.memset(v_sb[:, :, 65:66], 1.0)
                for grp in range(N_JB // 4):  # 4 j-blocks per psum slot
                    vslot = ps.tile([128, 4, 64], f32, tag="av", name="v_ps")
                    for j in range(4):
                        jb = grp * 4 + j
                        for cc in range(2):
                            nc.tensor.matmul(
                                out=vslot[:, j, :],
                                lhsT=x_r[cc][:, jb * JB:(jb + 1) * JB],
                                rhs=wv_r[cc][:, :],
                                start=(cc == 0), stop=(cc == 1))
                    for j in range(4):
                        jb = grp * 4 + j
                        nc.vector.scalar_tensor_tensor(
                            out=v_sb[:, jb, 0:32], in0=vslot[:, j, 0:32], scalar=1.0,
                            in1=bv_sb[:, 0:32], op0=MULT, op1=ADD)
                        nc.vector.scalar_tensor_tensor(
                            out=v_sb[:, jb, 33:65], in0=vslot[:, j, 32:64], scalar=1.0,
                            in1=bv_sb[:, 32:64], op0=MULT, op1=ADD)

            def qk_phase(dst, w_r, b_sb, h):
                # 8 i-blocks, 3 per [128,1536] psum slot
                for g0 in range(0, N_IB, 3):
                    blks = list(range(g0, min(g0 + 3, N_IB)))
                    slot = ps.tile([128, len(blks) * IB], f32, tag="ps3", name="qk_ps")
                    for bi, ib in enumerate(blks):
                        for cc in range(2):
                            nc.tensor.matmul(
                                out=slot[:, bi * IB:(bi + 1) * IB],
                                lhsT=w_r[h][cc][:, :],
                                rhs=x_r[cc][:, ib * IB:(ib + 1) * IB],
                                start=(cc == 0), stop=(cc == 1))
                    dsl = dst[h][:, g0 * IB:(g0 + len(blks)) * IB]
                    if h == 0:
                        # ScalarE is idle before the first wave
                        nc.scalar.add(out=dsl, in_=slot[:, :], add=b_sb[:, h:h + 1])
                    else:
                        # DVE has slack once waves are running
                        nc.vector.tensor_scalar_add(out=dsl, in0=slot[:, :],
                                                    scalar1=b_sb[:, h:h + 1])

            # V in [token, d] layout + ones columns: [128, 32jb, 66]
            v_sb = bp.tile([128, N_JB, 66], bf16, name="v_sb", tag="v_sb")
            qk_phase(kt, wk_r, bk_sb, 0)
            qk_phase(qt, wq_r, bq_sb, 0)
            v_phase()

            # ---- attention waves + projection -----------------------------
            # phase3(n) is emitted between i-block n+1's two head passes so
            # the PE never idles on the reciprocal chain (keeps HAM warm).
            pending_p3 = [None]

            def flush_p3():
                if pending_p3[0] is not None:
                    pending_p3[0]()
                    pending_p3[0] = None

            for n in range(N_IB):
                i0 = n * IB
                acc = wkp.tile([33, 1024], f32, tag="acc", name="acc")
                for h in range(2):
                    if n == 0 and h == 1:
                        qk_phase(kt, wk_r, bk_sb, 1)
                        qk_phase(qt, wq_r, bq_sb, 1)
                    if h == 1:
                        flush_p3()
                    av = ps.tile([33, IB], f32, tag="av", name="av_ps")
                    prev = None  # (ew, jbs) of previous wave

                    def emit_av(prev):
                        ew_p, jbs_p = prev
                        for r, jb in enumerate(jbs_p):
                            nc.tensor.matmul(
                                out=av[:, :],
                                lhsT=v_sb[:, jb, 33 * h:33 * h + 33],
                                rhs=ew_p[:, r * IB:(r + 1) * IB],
                                start=(jb == 0), stop=(jb == N_JB - 1),
                                tile_position=(0, 0))

                    for w in range(N_WAVE):
                        jbs = list(range(w * WAVE_JB, min((w + 1) * WAVE_JB, N_JB)))
                        slot = ps.tile([128, len(jbs) * IB], f32, tag="ps3", name="wave_ps")
                        for r, jb in enumerate(jbs):
                            nc.tensor.matmul(
                                out=slot[:, r * IB:(r + 1) * IB],
                                lhsT=kt[h][32 * r:32 * r + 32, jb * JB:(jb + 1) * JB],
                                rhs=qt[h][32 * r:32 * r + 32, i0:i0 + IB],
                                start=True, stop=True, tile_position=(32 * r, 0))
                        if prev is not None:
                            emit_av(prev)
                        ew = ep.tile([128, len(jbs) * IB], bf16, tag="ew", name="ew")
                        nc.scalar.activation(out=ew[:], in_=slot[:, :], func=Exp, scale=SCALE)
                        prev = (ew, jbs)
                    emit_av(prev)
                    nc.vector.tensor_copy(out=acc[:, h * IB:(h + 1) * IB], in_=av[:, :])

                # projection for this i-block; reciprocal right away (DVE),
                # PE-side matmuls deferred into the next i-block's waves
                recip = wkp.tile([33, 1024], f32, tag="recip", name="recip", bufs=1)
                nc.vector.tensor_copy(out=recip[0:1, :], in_=acc[32:33, :])
                recipf = wkp.tile([1, 1024], f32, tag="recipf", name="recipf", bufs=1)
                nc.vector.reciprocal_approx_fast(out=recipf[0:1, :], in_=recip[0:1, :])
                bc_sb = wkp.tile([32, 1024], f32, tag="bc_sb", name="bc_sb", bufs=1)
                nc.gpsimd.partition_broadcast(bc_sb[:, :], recipf[0:1, :])

                def phase3(n=n, i0=i0, acc=acc, bc_sb=bc_sb):
                    norm = wkp.tile([32, 1024], f32r, tag="norm", name="norm", bufs=1)
                    nc.vector.tensor_tensor(out=norm[:, :], in0=bc_sb[:, :],
                                            in1=acc[0:32, :], op=MULT)
                    for cc in range(2):
                        pj = ps.tile([128, IB], f32, tag="av", name="pj_ps")
                        for h in range(2):
                            nc.tensor.matmul(
                                out=pj[:, :],
                                lhsT=wo_r[:, h * C + cc * 128: h * C + (cc + 1) * 128],
                                rhs=norm[:, h * IB:(h + 1) * IB],
                                start=(h == 0), stop=(h == 1), tile_position=(0, 0))
                        y = wkp.tile([128, IB], f32, tag="y", name="y")
                        nc.vector.tensor_scalar_add(out=y[:], in0=pj[:, :],
                                                    scalar1=bo_sb[:, cc:cc + 1])
                        nc.sync.dma_start(out=o[cc * 128:(cc + 1) * 128, i0:i0 + IB], in_=y[:])

                pending_p3[0] = phase3
            flush_p3()

    nc.finalize()
    return nc


def _get_compiled():
    global _COMPILED
    if _COMPILED is None:
        _COMPILED = _build()
    return _COMPILED


def _make_in_maps(x, w_qkv, b_qkv, w_out, b_out):
    x = np.asarray(x, dtype=np.float32)
    w_qkv = np.asarray(w_qkv, dtype=np.float32)
    b_qkv = np.asarray(b_qkv, dtype=np.float32)
    w_out = np.asarray(w_out, dtype=np.float32)
    b_out = np.asarray(b_out, dtype=np.float32)

    xf = x.reshape(B, C, N)
    in_maps = []
    for core in range(NCORES):
        b = core // 4
        q = core % 4
        heads = (2 * q, 2 * q + 1)

        def rep_cols(w_slice):  # [C,32] -> [C,128] (4 replicas)
            return np.ascontiguousarray(np.tile(w_slice, (1, 4)))

        wq = np.stack([rep_cols(w_qkv[:, 32 * h:32 * h + 32]) for h in heads])
        wk = np.stack([rep_cols(w_qkv[:, C + 32 * h:C + 32 * h + 32]) for h in heads])
        wv_ = np.concatenate([w_qkv[:, 2 * C + 32 * h:2 * C + 32 * h + 32] for h in heads], axis=1)
        bq = np.stack([np.tile(b_qkv[32 * h:32 * h + 32], 4) for h in heads])
        bk = np.stack([np.tile(b_qkv[C + 32 * h:C + 32 * h + 32], 4) for h in heads])
        bv = np.concatenate([b_qkv[2 * C + 32 * h:2 * C + 32 * h + 32] for h in heads])
        bv_rep = np.tile(bv[None, :], (128, 1))
        wo_ = np.concatenate([w_out[32 * h:32 * h + 32, :] for h in heads], axis=1)
        in_maps.append({
            "xb": np.ascontiguousarray(xf[b]),
            "wq_rep": wq,
            "wk_rep": wk,
            "wv": np.ascontiguousarray(wv_),
            "bq_rep": np.ascontiguousarray(bq),
            "bk_rep": np.ascontiguousarray(bk),
            "bv_rep": np.ascontiguousarray(bv_rep),
            "wo": np.ascontiguousarray(wo_),
            "bo4": np.ascontiguousarray(b_out / 4.0),
        })
    return in_maps


def kernel(x, w_qkv, b_qkv, w_out, b_out, _trace=False, _trace_kwargs=None):
    from concourse.bass_utils import run_bass_kernel_spmd

    nc = _get_compiled()
    in_maps = _make_in_maps(x, w_qkv, b_qkv, w_out, b_out)
    res = run_bass_kernel_spmd(nc, in_maps, list(range(NCORES)),
                               trace=_trace, **(_trace_kwargs or {}))
    parts = [res.results[c]["o"] for c in range(NCORES)]
    out = np.empty((B, C, N), dtype=np.float32)
    for b in range(B):
        out[b] = parts[4 * b] + parts[4 * b + 1] + parts[4 * b + 2] + parts[4 * b + 3]
    result = out.reshape(B, C, HW, HW)
    if _trace:
        return result, res
    return result
